# revision 35
# baseline (speedup 1.0000x reference)
"""CapsuleMaxPooling Trainium2 kernel.

Problem: inp [B=32, C=32, H=64, W=64, D=8] f32, kernel_size k=2.
For each 2x2 spatial window pick the capsule vector (length D=8) with the
largest squared L2 norm (first-max tie-break) -> out [B, C, 32, 32, 8].

Strategy (fully data-parallel, shard B across 8 cores; per core the shard is
viewed as rows r=(b, c, hk) of 1024 contiguous floats = (dh, wk, dw, d),
i.e. both H-rows of all windows in that row; 32 row-tiles of 128 partitions):
  - ACT: sq = x^2 (Square activation), plus the base copy of candidate D
    into the output tile (early, off the DVE critical path).
  - DVE: norms = grouped reduce_sum over d (groups of 8); 5-op tournament
    per group (wC = nC>=nD, wB = nB>=max(nC,nD), wA = nA>=max(nB,nC,nD));
    with the predication ORDER (D base, then C, then B, then A last) this
    yields exact first-argmax; 3x copy_predicated overwrite per batch.
    copy_predicated wants an integer mask: we hand it an int32 bitcast
    view of the f32 0.0/1.0 mask (1.0f = 0x3F800000 != 0) broadcast over
    d via a stride-0 inner dim.
  - GpSimd is deliberately NOT used: it shares SBUF read/write ports with
    DVE on Trn2, so any "offload" to it just steals DVE cycles 1:1
    (verified in perfetto: concurrent DVE slices stretch to GpSimd slice
    end). PE cannot help either (contraction is partition-only; on-chip
    transposes would need PSUM->SBUF copies costing more than they save).
  - HWDGE (nc.sync) DMAs, contiguous 4KB-per-partition chunks. Small
    batches at the schedule edges shorten pipeline ramp-in/ramp-out.
  - Emission is software-pipelined per engine (see comment in build_nc):
    each engine executes its stream in order, so selection work lags its
    group by 1 and output stores lag by 2, keeping DVE ~98% occupied and
    the input DMA stream never blocked.

Engine budget per core (cost model, cycle_t: DVE 1.04ns, ACT 0.83ns):
ACT = 32768 sq + 8192 copy ~ 38us; DVE = 32768 reduce + 24576 pred-copy +
6144 tournament + per-op fixed ~ 74us (the bottleneck, ~98% occupied);
DMA floor 20.97 MB @ ~360 GB/s ~ 58us; plus ~6us runtime preroll and
~4us tail barrier -> ~93us.
"""

import numpy as np

try:
    import concourse.bass as bass
except ImportError:  # pragma: no cover
    import sys

    sys.path.insert(0, "/opt/trn_rl_repo")
    import concourse.bass as bass

from concourse import bacc, mybir
from concourse.bass_utils import run_bass_kernel_spmd
from concourse.tile import TileContext

P = 128
N_CORES = 8
ROW_W = 1024  # (dh=2) * (wk=32) * (dw=2) * (d=8)
OUT_W = 256  # (wk=32) * (d=8)
# (row-tiles, reduce engine) per batch; tiles sum to R // P (= 32).
# 'v' = DVE tensor_reduce, 'g' = GpSimd avg-pool (sum/8; exact /8 keeps the
# argmax). Interleaved so neither engine falls behind the ~5.7us/batch DMA
# arrival rate: GpSimd pool = 5.7us per 4-tile batch, DVE carries tournament
# + predicated copies (~4.6us/batch) plus every few batches a reduce.
DEFAULT_SCHED = (
    (1, "v"),
    (1, "v"),
    (2, "v"),
    (2, "v"),
    (4, "v"),
    (4, "v"),
    (4, "v"),
    (4, "v"),
    (4, "v"),
    (4, "v"),
    (1, "v"),
    (1, "v"),
)


def _bcs(w, q0, qn, n):
    """Slice mask tile w [P, GTB, 32] rows [q0:q0+qn], viewed as int32
    [P, qn, 32, n] via a bitcast + stride-0 inner dim (copy_predicated
    wants an integer mask; 1.0f = 0x3F800000 != 0)."""
    a = w[:, q0 : q0 + qn].bitcast(mybir.dt.int32)
    return bass.AP(tensor=a.tensor, offset=a.offset, ap=[*a.ap, [0, n]])


def build_nc(R=4096, sched=DEFAULT_SCHED, GM=2):
    """Build the per-core Bass program. R = rows (b,c,hk) per core."""
    f32 = mybir.dt.float32
    nc = bacc.Bacc(None, target_bir_lowering=False)
    x = nc.dram_tensor("x", [R, ROW_W], f32, kind="ExternalInput")
    y = nc.dram_tensor("y", [R, OUT_W], f32, kind="ExternalOutput")
    assert sum(tb for tb, _ in sched) * P == R
    # group consecutive batches for the mask stage (amortizes small-op cost)
    groups = [list(sched[i : i + GM]) for i in range(0, len(sched), GM)]

    with TileContext(nc) as tc:
        with (
            tc.tile_pool(name="xp", bufs=6) as xp,
            tc.tile_pool(name="sqp", bufs=3) as sqp,
            tc.tile_pool(name="normp", bufs=3) as normp,
            tc.tile_pool(name="maskp", bufs=2) as maskp,
            tc.tile_pool(name="outp", bufs=7) as outp,
        ):
            # Each hardware engine executes its instruction stream IN ORDER:
            # an instruction whose semaphore wait is unmet stalls everything
            # emitted after it on that engine (and on SP that includes all
            # later input DMAs). So emission is software-pipelined with
            # per-engine lag: for group k we emit loads+squares(k) first,
            # then output stores of group k-2 (SP reaches them long after
            # their producers finished), then the selection (tournament +
            # predicated copies) of group k-1 on DVE (its norms finished
            # during group k-1), then the reduces(k) (DVE reduces run while
            # ACT is still squaring group k).
            tile0 = 0
            pending_sel = []  # [(grp, norms, xts, ots, qoff)]
            pending_outs = []  # [[(r0, tb, ot), ...]] per group

            def emit_sel(grp, norms, xts, ots, qoff):
                gtb = sum(tb for tb, _ in grp)
                # 5-op tournament on the whole group's norms. With the
                # predication order D base -> C -> B -> A(last), candidate X's
                # mask only needs X >= max(candidates AFTER X in that order):
                #   wC = nC >= nD, wB = nB >= max(nC,nD), wA = nA >= max(B,C,D)
                # Ties fire the earlier-index overwrite too (it runs later),
                # so exact first-argmax is preserved with 2 maxes + 3 is_ge.
                nr = norms.rearrange(
                    "p j (dh wk dw) -> p j dh wk dw", dh=2, dw=2
                )
                nA = nr[:, :, 0, :, 0]
                nB = nr[:, :, 0, :, 1]
                nC = nr[:, :, 1, :, 0]
                nD = nr[:, :, 1, :, 1]

                wC = maskp.tile([P, gtb, 32], f32, tag="wC")
                nc.vector.tensor_tensor(wC, nC, nD, op=mybir.AluOpType.is_ge)
                h2 = maskp.tile([P, gtb, 32], f32, tag="h2")
                nc.vector.tensor_tensor(h2, nC, nD, op=mybir.AluOpType.max)
                wB = maskp.tile([P, gtb, 32], f32, tag="wB")
                nc.vector.tensor_tensor(wB, nB, h2, op=mybir.AluOpType.is_ge)
                M = maskp.tile([P, gtb, 32], f32, tag="M")
                nc.vector.tensor_tensor(M, nB, h2, op=mybir.AluOpType.max)
                wA = maskp.tile([P, gtb, 32], f32, tag="wA")
                nc.vector.tensor_tensor(wA, nA, M, op=mybir.AluOpType.is_ge)

                outs = []
                for qi, (tb, red) in enumerate(grp):
                    xt = xts[qi]
                    ot, r0 = ots[qi]
                    xr = xt.rearrange(
                        "p j (dh wk dw d) -> p j dh wk dw d", dh=2, dw=2, d=8
                    )
                    Av = xr[:, :, 0, :, 0, :]
                    Bv = xr[:, :, 0, :, 1, :]
                    Cv = xr[:, :, 1, :, 0, :]
                    q0 = qoff[qi]
                    nc.vector.copy_predicated(ot, _bcs(wC, q0, tb, 8), Cv)
                    nc.vector.copy_predicated(ot, _bcs(wB, q0, tb, 8), Bv)
                    nc.vector.copy_predicated(ot, _bcs(wA, q0, tb, 8), Av)
                    outs.append((r0, tb, ot))
                return outs

            def emit_outs(outs):
                for r0, tb, ot in outs:
                    nc.sync.dma_start(
                        out=y[r0 : r0 + tb * P, :].rearrange(
                            "(j p) c -> p j c", p=P
                        ),
                        in_=ot.rearrange("p j w d -> p j (w d)"),
                    )

            for grp in groups:
                gtb = sum(tb for tb, _ in grp)
                norms = normp.tile([P, gtb, 128], f32, tag="norms")
                xts = []
                sqs = []
                ots = []
                qoff = [0]
                for tb, red in grp:
                    r0 = tile0 * P
                    xt = xp.tile([P, tb, ROW_W], f32, tag="xt")
                    xts.append(xt)
                    nc.sync.dma_start(
                        out=xt,
                        in_=x[r0 : r0 + tb * P, :].rearrange(
                            "(j p) c -> p j c", p=P
                        ),
                    )
                    sq = sqp.tile([P, tb, ROW_W], f32, tag="sq")
                    nc.scalar.square(sq, xt)
                    sqs.append(sq)
                    ot = outp.tile([P, tb, 32, 8], f32, tag="ot")
                    ots.append((ot, r0))
                    qoff.append(qoff[-1] + tb)
                    tile0 += tb

                if len(pending_outs) >= 2:
                    emit_outs(pending_outs.pop(0))
                if pending_sel:
                    pending_outs.append(emit_sel(*pending_sel.pop(0)))

                for qi, (tb, red) in enumerate(grp):
                    nslice = norms[:, qoff[qi] : qoff[qi] + tb]
                    sqg = sqs[qi].rearrange("p j (gr d) -> p j gr d", d=8)
                    # NOTE: GpSimd is NOT used anywhere — it shares SBUF ports
                    # with DVE, so offloading to it just steals DVE cycles.
                    nc.vector.tensor_reduce(
                        nslice,
                        sqg,
                        axis=mybir.AxisListType.X,
                        op=mybir.AluOpType.add,
                    )

                for qi, (tb, red) in enumerate(grp):
                    xr = xts[qi].rearrange(
                        "p j (dh wk dw d) -> p j dh wk dw d", dh=2, dw=2, d=8
                    )
                    nc.scalar.copy(ots[qi][0], xr[:, :, 1, :, 1, :])

                pending_sel.append((grp, norms, xts, ots, qoff))

            while pending_sel or pending_outs:
                if pending_outs:
                    emit_outs(pending_outs.pop(0))
                if pending_sel:
                    pending_outs.append(emit_sel(*pending_sel.pop(0)))
    nc.compile()
    return nc


_NC_CACHE = {}


def _get_nc(R):
    if R not in _NC_CACHE:
        _NC_CACHE[R] = build_nc(R)
    return _NC_CACHE[R]


def kernel(inp, kernel_size):
    inp = np.asarray(inp)
    k = int(np.asarray(kernel_size))
    assert k == 2, f"kernel hardcoded for kernel_size=2, got {k}"
    B, C, H, W, D = inp.shape
    assert (B, C, H, W, D) == (32, 32, 64, 64, 8), inp.shape
    Hk, Wk = H // k, W // k

    bs = B // N_CORES  # 4 batches per core
    R = bs * C * Hk  # 4096 rows per core
    nc = _get_nc(R)

    in_maps = []
    for c in range(N_CORES):
        shard = np.ascontiguousarray(inp[c * bs : (c + 1) * bs]).reshape(R, ROW_W)
        in_maps.append({"x": shard})

    res = run_bass_kernel_spmd(nc, in_maps, list(range(N_CORES)))
    out = np.concatenate(
        [r["y"].reshape(bs, C, Hk, Wk, D) for r in res.results], axis=0
    )
    return out


# revision 36
# speedup vs baseline: 1.0162x; 1.0162x over previous
"""CapsuleMaxPooling Trainium2 kernel.

Problem: inp [B=32, C=32, H=64, W=64, D=8] f32, kernel_size k=2.
For each 2x2 spatial window pick the capsule vector (length D=8) with the
largest squared L2 norm (first-max tie-break) -> out [B, C, 32, 32, 8].

Strategy (fully data-parallel, shard B across 8 cores; per core the shard is
viewed as rows r=(b, c, hk) of 1024 contiguous floats = (dh, wk, dw, d),
i.e. both H-rows of all windows in that row; 32 row-tiles of 128 partitions):
  - ACT: sq = x^2 (Square activation), plus the base copy of candidate D
    into the output tile (early, off the DVE critical path).
  - DVE: norms = grouped reduce_sum over d (groups of 8); 5-op tournament
    per group (wC = nC>=nD, wB = nB>=max(nC,nD), wA = nA>=max(nB,nC,nD));
    with the predication ORDER (D base, then C, then B, then A last) this
    yields exact first-argmax; 3x copy_predicated overwrite per batch.
    copy_predicated wants an integer mask: we hand it an int32 bitcast
    view of the f32 0.0/1.0 mask (1.0f = 0x3F800000 != 0) broadcast over
    d via a stride-0 inner dim.
  - GpSimd is deliberately NOT used: it shares SBUF read/write ports with
    DVE on Trn2, so any "offload" to it just steals DVE cycles 1:1
    (verified in perfetto: concurrent DVE slices stretch to GpSimd slice
    end). PE cannot help either (contraction is partition-only; on-chip
    transposes would need PSUM->SBUF copies costing more than they save).
  - HWDGE (nc.sync) DMAs, contiguous 4KB-per-partition chunks. Small
    batches at the schedule edges shorten pipeline ramp-in/ramp-out.
  - Emission is software-pipelined per engine (see comment in build_nc):
    each engine executes its stream in order, so selection work lags its
    group by 1 and output stores lag by 2, keeping DVE ~98% occupied and
    the input DMA stream never blocked.

Engine budget per core (cost model, cycle_t: DVE 1.04ns, ACT 0.83ns):
ACT = 32768 sq + 8192 copy ~ 38us; DVE = 32768 reduce + 24576 pred-copy +
6144 tournament + per-op fixed ~ 74us (the bottleneck, ~98% occupied);
DMA floor 20.97 MB @ ~360 GB/s ~ 58us; plus ~6us runtime preroll and
~4us tail barrier -> ~93us.
"""

import numpy as np

try:
    import concourse.bass as bass
except ImportError:  # pragma: no cover
    import sys

    sys.path.insert(0, "/opt/trn_rl_repo")
    import concourse.bass as bass

from concourse import bacc, mybir
from concourse.bass_utils import run_bass_kernel_spmd
from concourse.tile import TileContext

P = 128
N_CORES = 8
ROW_W = 1024  # (dh=2) * (wk=32) * (dw=2) * (d=8)
OUT_W = 256  # (wk=32) * (d=8)
# (row-tiles, reduce engine) per batch; tiles sum to R // P (= 32).
# 'v' = DVE tensor_reduce, 'g' = GpSimd avg-pool (sum/8; exact /8 keeps the
# argmax). Interleaved so neither engine falls behind the ~5.7us/batch DMA
# arrival rate: GpSimd pool = 5.7us per 4-tile batch, DVE carries tournament
# + predicated copies (~4.6us/batch) plus every few batches a reduce.
DEFAULT_SCHED = (
    (1, "v"),
    (1, "v"),
    (2, "v"),
    (2, "v"),
    (4, "v"),
    (4, "v"),
    (4, "v"),
    (4, "v"),
    (4, "v"),
    (4, "v"),
    (1, "v"),
    (1, "v"),
)


def _bcs(w, q0, qn, n):
    """Slice mask tile w [P, GTB, 32] rows [q0:q0+qn], viewed as int32
    [P, qn, 32, n] via a bitcast + stride-0 inner dim (copy_predicated
    wants an integer mask; 1.0f = 0x3F800000 != 0)."""
    a = w[:, q0 : q0 + qn].bitcast(mybir.dt.int32)
    return bass.AP(tensor=a.tensor, offset=a.offset, ap=[*a.ap, [0, n]])


def build_nc(R=4096, sched=DEFAULT_SCHED, GM=2):
    """Build the per-core Bass program. R = rows (b,c,hk) per core."""
    f32 = mybir.dt.float32
    nc = bacc.Bacc(None, target_bir_lowering=False)
    x = nc.dram_tensor("x", [R, ROW_W], f32, kind="ExternalInput")
    y = nc.dram_tensor("y", [R, OUT_W], f32, kind="ExternalOutput")
    assert sum(tb for tb, _ in sched) * P == R
    # group consecutive batches for the mask stage (amortizes small-op cost)
    groups = [list(sched[i : i + GM]) for i in range(0, len(sched), GM)]

    with TileContext(nc) as tc:
        with (
            tc.tile_pool(name="xp", bufs=6) as xp,
            tc.tile_pool(name="sqp", bufs=3) as sqp,
            tc.tile_pool(name="normp", bufs=3) as normp,
            tc.tile_pool(name="maskp", bufs=2) as maskp,
            tc.tile_pool(name="outp", bufs=7) as outp,
        ):
            # Each hardware engine executes its instruction stream IN ORDER:
            # an instruction whose semaphore wait is unmet stalls everything
            # emitted after it on that engine (and on SP that includes all
            # later input DMAs). So emission is software-pipelined with
            # per-engine lag: for group k we emit loads+squares(k) first,
            # then output stores of group k-2 (SP reaches them long after
            # their producers finished), then the selection (tournament +
            # predicated copies) of group k-1 on DVE (its norms finished
            # during group k-1), then the reduces(k) (DVE reduces run while
            # ACT is still squaring group k).
            tile0 = 0
            pending_sel = []  # [(grp, norms, xts, ots, qoff)]
            pending_outs = []  # [[(r0, tb, ot), ...]] per group

            def emit_sel(grp, norms, xts, ots, qoff):
                gtb = sum(tb for tb, _ in grp)
                # 5-op tournament on the whole group's norms. With the
                # predication order D base -> C -> B -> A(last), candidate X's
                # mask only needs X >= max(candidates AFTER X in that order):
                #   wC = nC >= nD, wB = nB >= max(nC,nD), wA = nA >= max(B,C,D)
                # Ties fire the earlier-index overwrite too (it runs later),
                # so exact first-argmax is preserved with 2 maxes + 3 is_ge.
                nr = norms.rearrange(
                    "p j (dh wk dw) -> p j dh wk dw", dh=2, dw=2
                )
                nA = nr[:, :, 0, :, 0]
                nB = nr[:, :, 0, :, 1]
                nC = nr[:, :, 1, :, 0]
                nD = nr[:, :, 1, :, 1]

                wC = maskp.tile([P, gtb, 32], f32, tag="wC")
                nc.vector.tensor_tensor(wC, nC, nD, op=mybir.AluOpType.is_ge)
                h2 = maskp.tile([P, gtb, 32], f32, tag="h2")
                nc.vector.tensor_tensor(h2, nC, nD, op=mybir.AluOpType.max)
                wB = maskp.tile([P, gtb, 32], f32, tag="wB")
                nc.vector.tensor_tensor(wB, nB, h2, op=mybir.AluOpType.is_ge)
                M = maskp.tile([P, gtb, 32], f32, tag="M")
                nc.vector.tensor_tensor(M, nB, h2, op=mybir.AluOpType.max)
                wA = maskp.tile([P, gtb, 32], f32, tag="wA")
                nc.vector.tensor_tensor(wA, nA, M, op=mybir.AluOpType.is_ge)

                outs = []
                for qi, (tb, red) in enumerate(grp):
                    xt = xts[qi]
                    ot, r0 = ots[qi]
                    xr = xt.rearrange(
                        "p j (dh wk dw d) -> p j dh wk dw d", dh=2, dw=2, d=8
                    )
                    Av = xr[:, :, 0, :, 0, :]
                    Bv = xr[:, :, 0, :, 1, :]
                    Cv = xr[:, :, 1, :, 0, :]
                    q0 = qoff[qi]
                    nc.vector.copy_predicated(ot, _bcs(wC, q0, tb, 8), Cv)
                    nc.vector.copy_predicated(ot, _bcs(wB, q0, tb, 8), Bv)
                    nc.vector.copy_predicated(ot, _bcs(wA, q0, tb, 8), Av)
                    outs.append((r0, tb, ot))
                return outs

            def emit_outs(outs):
                for r0, tb, ot in outs:
                    nc.sync.dma_start(
                        out=y[r0 : r0 + tb * P, :].rearrange(
                            "(j p) c -> p j c", p=P
                        ),
                        in_=ot.rearrange("p j w d -> p j (w d)"),
                    )

            for grp in groups:
                gtb = sum(tb for tb, _ in grp)
                norms = normp.tile([P, gtb, 128], f32, tag="norms")
                xts = []
                sqs = []
                ots = []
                qoff = [0]
                for tb, red in grp:
                    r0 = tile0 * P
                    xt = xp.tile([P, tb, ROW_W], f32, tag="xt")
                    xts.append(xt)
                    nc.sync.dma_start(
                        out=xt,
                        in_=x[r0 : r0 + tb * P, :].rearrange(
                            "(j p) c -> p j c", p=P
                        ),
                    )
                    sq = sqp.tile([P, tb, ROW_W], f32, tag="sq")
                    if tile0 == 0:
                        # very first batch: square on DVE (tensor_tensor mult)
                        # instead of ACT — no dependency on the activation
                        # table load, so DVE starts ~2.5us earlier for +1.1us
                        # of added DVE work, shortening the pipeline head
                        nc.vector.tensor_tensor(
                            sq, xt, xt, op=mybir.AluOpType.mult
                        )
                    else:
                        nc.scalar.square(sq, xt)
                    sqs.append(sq)
                    ot = outp.tile([P, tb, 32, 8], f32, tag="ot")
                    ots.append((ot, r0))
                    qoff.append(qoff[-1] + tb)
                    tile0 += tb

                if len(pending_outs) >= 2:
                    emit_outs(pending_outs.pop(0))
                if pending_sel:
                    pending_outs.append(emit_sel(*pending_sel.pop(0)))

                for qi, (tb, red) in enumerate(grp):
                    nslice = norms[:, qoff[qi] : qoff[qi] + tb]
                    sqg = sqs[qi].rearrange("p j (gr d) -> p j gr d", d=8)
                    # NOTE: GpSimd is NOT used anywhere — it shares SBUF ports
                    # with DVE, so offloading to it just steals DVE cycles.
                    nc.vector.tensor_reduce(
                        nslice,
                        sqg,
                        axis=mybir.AxisListType.X,
                        op=mybir.AluOpType.add,
                    )

                for qi, (tb, red) in enumerate(grp):
                    xr = xts[qi].rearrange(
                        "p j (dh wk dw d) -> p j dh wk dw d", dh=2, dw=2, d=8
                    )
                    nc.scalar.copy(ots[qi][0], xr[:, :, 1, :, 1, :])

                pending_sel.append((grp, norms, xts, ots, qoff))

            while pending_sel or pending_outs:
                if pending_outs:
                    emit_outs(pending_outs.pop(0))
                if pending_sel:
                    pending_outs.append(emit_sel(*pending_sel.pop(0)))
    nc.compile()
    return nc


_NC_CACHE = {}


def _get_nc(R):
    if R not in _NC_CACHE:
        _NC_CACHE[R] = build_nc(R)
    return _NC_CACHE[R]


def kernel(inp, kernel_size):
    inp = np.asarray(inp)
    k = int(np.asarray(kernel_size))
    assert k == 2, f"kernel hardcoded for kernel_size=2, got {k}"
    B, C, H, W, D = inp.shape
    assert (B, C, H, W, D) == (32, 32, 64, 64, 8), inp.shape
    Hk, Wk = H // k, W // k

    bs = B // N_CORES  # 4 batches per core
    R = bs * C * Hk  # 4096 rows per core
    nc = _get_nc(R)

    in_maps = []
    for c in range(N_CORES):
        shard = np.ascontiguousarray(inp[c * bs : (c + 1) * bs]).reshape(R, ROW_W)
        in_maps.append({"x": shard})

    res = run_bass_kernel_spmd(nc, in_maps, list(range(N_CORES)))
    out = np.concatenate(
        [r["y"].reshape(bs, C, Hk, Wk, D) for r in res.results], axis=0
    )
    return out
